# revision 16
# baseline (speedup 1.0000x reference)
"""DeepAR 2-layer LSTM (B=16, L_IN=96, L_OUT=24, N=320, H=128) on 8 TRN2 cores.

Strategy (data-parallel over B*N=5120 rows, 640 rows/core):
  - Layout: hidden/gate index on SBUF partitions, batch rows on the free dim.
    h is [128, 640] bf16; cell state c and the whole gate path in fp32.
  - All four gates use the SIGMOID table: tanh(g) = 2*sigmoid(2g) - 1 with the
    2x folded into the g-gate rows of the weights host-side. This lets the
    four gates share ACTIVATE instructions.
  - Gates land in "quad" PSUM tiles: quadA [128, 2048] holds all 4 gates for
    row-columns 0:512 (one PSUM bank per gate -> accumulation groups never
    share a bank); quadB [128, 512] holds columns 512:640 (4 gates x 128 in
    one bank -> per-gate matmul groups are emitted closed, never interleaved).
  - One sigmoid per quad (2 per layer step) with a strided OUTPUT access
    pattern that lands gate-contiguous in a [128, 2560] fp32 SBUF tile, so
    the cell-update elementwise ops run on contiguous 640-wide slices.
  - tanh(c) reads a cross-layer c-pair tile [c0(t) | c1(t-1)]: 2 instrs,
    split at column 512 so h0's first 512 columns unblock the next matmuls.
  - ACT instrs/step: 6 (~6.8us busy) vs 12 in the per-gate baseline.
  - L0 input transform folded through the embedding: x_eff[t] = [tgt_t,
    cov_{t+1}, 1] (6 values/row incl bias feature) with W0eff = [Wih0[:, :E]
    @ We, Wih0[:, E:], b0] -> K=6 matmuls packed 4-way via tile_position.
  - Engines: PE matmuls; ACT sigmoids (bottleneck); DVE p/c/h muls; GPSIMD
    does q=sf*c_prev and the g~=2*sg-1 transform.
"""

import os

import numpy as np

B, L_IN, L_OUT, N_SER, COV = 16, 96, 24, 320, 4
E, H = 64, 128
T_STEPS = 119  # L_IN + L_OUT - 1
NCORES = 8
BN = B * N_SER          # 5120
R = BN // NCORES        # 640 rows per core
RA = 512                # quadA columns (one bank per gate)
RB = R - RA             # quadB columns
G4 = 4 * H              # 512 gates
KX = 6                  # x features: tgt, 4 cov, ones(bias)

_PROGRAM_CACHE: dict = {}

_PERM_ORDER = (1, 2, 0, 3)  # torch (i,f,g,o) -> (f, g, i, o)


def _gate_perm() -> np.ndarray:
    idx = np.arange(G4).reshape(4, H)
    return np.concatenate([idx[g] for g in _PERM_ORDER])


def _build_program(t_steps: int, scan_steps: int | None = None, repeat: int = 1,
                   with_b1: bool = False):
    import concourse.bacc as bacc
    import concourse.mybir as mybir
    import concourse.tile as tile

    f32 = mybir.dt.float32
    bf16 = mybir.dt.bfloat16
    f16 = mybir.dt.float16
    AF = mybir.ActivationFunctionType
    ALU = mybir.AluOpType

    nc = bacc.Bacc()

    xrep_d = nc.declare_dram_parameter("xrep", [t_steps, 4 * KX, R], bf16, isOutput=False)
    # weights transposed [H, 4H], gate blocks in (f, g, i, o) order, g rows 2x
    whh0t_d = nc.declare_dram_parameter("whh0t", [H, G4], bf16, isOutput=False)
    w0rep_d = nc.declare_dram_parameter("w0rep", [128, 128], bf16, isOutput=False)
    wih1t_d = nc.declare_dram_parameter("wih1t", [H, G4], bf16, isOutput=False)
    whh1t_d = nc.declare_dram_parameter("whh1t", [H, G4], bf16, isOutput=False)
    b1r_d = (
        nc.declare_dram_parameter("b1r", [1, G4], bf16, isOutput=False)
        if with_b1
        else None
    )
    wht_d = nc.declare_dram_parameter("wht", [H, 2], bf16, isOutput=False)
    bh2_d = nc.declare_dram_parameter("bh2", [1, 2], bf16, isOutput=False)
    if scan_steps is None:
        scan_steps = t_steps
    n_hist = min(L_OUT, scan_steps)
    out_d = nc.declare_dram_parameter("out", [n_hist, 2, R], f32, isOutput=True)

    with tile.TileContext(nc) as tc:
        with (
            tc.tile_pool(name="consts", bufs=1) as consts,
            tc.tile_pool(name="xin", bufs=5) as xin,
            tc.tile_pool(name="gq", bufs=2, space="PSUM") as gq,
            tc.tile_pool(name="sig", bufs=3) as sigp,
            tc.tile_pool(name="tmp", bufs=6) as tmpp,
            tc.tile_pool(name="cpp", bufs=3) as cpp,
            tc.tile_pool(name="tcp", bufs=3) as tcp,
            tc.tile_pool(name="h0p", bufs=4) as h0p,
            tc.tile_pool(name="h1p", bufs=4) as h1p,
            tc.tile_pool(name="hist", bufs=n_hist + 1) as histp,
            tc.tile_pool(name="stage", bufs=1) as stagep,
        ):
            # ---- load constants ----
            whh0t = consts.tile([H, G4], bf16)
            nc.sync.dma_start(out=whh0t[:], in_=whh0t_d[:])
            w0rep = consts.tile([128, 128], bf16)
            nc.sync.dma_start(out=w0rep[:], in_=w0rep_d[:])
            wih1t = consts.tile([H, G4], bf16)
            nc.sync.dma_start(out=wih1t[:], in_=wih1t_d[:])
            whh1t = consts.tile([H, G4], bf16)
            nc.sync.dma_start(out=whh1t[:], in_=whh1t_d[:])
            if with_b1:
                b1r = consts.tile([1, G4], bf16)
                nc.sync.dma_start(out=b1r[:], in_=b1r_d[:])
            wht = consts.tile([H, 2], bf16)
            nc.sync.dma_start(out=wht[:], in_=wht_d[:])
            bh2t = consts.tile([1, 2], bf16)
            nc.sync.dma_start(out=bh2t[:], in_=bh2_d[:])
            ones = consts.tile([1, R], bf16)
            nc.vector.memset(ones[:], 1.0)

            def _scan_body():
                # ---- initial states ----
                h0 = h0p.tile([H, R], bf16)
                nc.vector.memset(h0[:], 0.0)
                h1 = h1p.tile([H, R], bf16)
                nc.vector.memset(h1[:], 0.0)
                # c-pair tiles: pair(t) = [c0(t) | c1(t-1)]
                cp_prev = cpp.tile([H, 2 * R], f32, tag="cp", name="cp_init0")
                nc.vector.memset(cp_prev[:, 0:R], 0.0)      # c0(-1)
                cp_cur = cpp.tile([H, 2 * R], f32, tag="cp", name="cp_init1")
                nc.vector.memset(cp_cur[:, R : 2 * R], 0.0)  # c1(-1)

                def in_mm(qt, g, lo, w, col, xt, layer0, start, stop):
                    """input-part matmul for gate g into quad column block."""
                    s = slice(col, col + w)
                    if layer0:
                        nc.tensor.matmul(
                            qt[:, s],
                            lhsT=w0rep[32 * g : 32 * g + KX, :],
                            rhs=xt[32 * g : 32 * g + KX, lo : lo + w],
                            start=start,
                            stop=stop,
                            tile_position=(32 * g, 0),
                        )
                    else:
                        nc.tensor.matmul(
                            qt[:, s],
                            lhsT=wih1t[:, g * H : (g + 1) * H],
                            rhs=xt[:, lo : lo + w],
                            start=start,
                            stop=stop,
                        )

                def rec_mm(qt, g, lo, w, col, h_in, wt, start, stop):
                    s = slice(col, col + w)
                    nc.tensor.matmul(
                        qt[:, s],
                        lhsT=wt[:, g * H : (g + 1) * H],
                        rhs=h_in[:, lo : lo + w],
                        start=start,
                        stop=stop,
                    )

                def bias_mm(qt, g, lo, w, col):
                    s = slice(col, col + w)
                    nc.tensor.matmul(
                        qt[:, s],
                        lhsT=b1r[0:1, g * H : (g + 1) * H],
                        rhs=ones[0:1, lo : lo + w],
                        start=False,
                        stop=True,
                    )

                def layer_mms(xt, h_in, wt, layer0):
                    """Emit matmuls for one layer step -> (quadA, quadB).

                    quadA: gates at 512-col offsets (bank-exclusive
                    accumulation groups). The operand that is available
                    EARLIEST goes first so the PE can stream it while the
                    other is still being computed: L0's x-part (xt is
                    prefetched), L1's recurrent part (h1(t-1) is old).
                    quadB: one bank shared by 4 gates -> each gate's group
                    closes before the next opens."""
                    qa = gq.tile([H, 4 * RA], f32, tag="quad", name="qa")
                    qb = gq.tile([H, 4 * RB], f32, tag="quad", name="qb")
                    use_b1 = with_b1 and not layer0
                    if layer0:
                        for g in range(4):
                            in_mm(qa, g, 0, RA, g * RA, xt, True, True, False)
                        for g in range(4):
                            rec_mm(qa, g, 0, RA, g * RA, h_in, wt, False,
                                   not use_b1)
                            if use_b1:
                                bias_mm(qa, g, 0, RA, g * RA)
                        for g in range(4):
                            in_mm(qb, g, RA, RB, g * RB, xt, True, True, False)
                            rec_mm(qb, g, RA, RB, g * RB, h_in, wt, False,
                                   not use_b1)
                            if use_b1:
                                bias_mm(qb, g, RA, RB, g * RB)
                    else:
                        # L1: recurrent part first (h1(t-1) ready long before
                        # h0(t)); input part = Wih1 @ h0 closes the group
                        for g in range(4):
                            rec_mm(qa, g, 0, RA, g * RA, h_in, wt, True, False)
                        for g in range(4):
                            in_mm(qa, g, 0, RA, g * RA, xt, False, False,
                                  not use_b1)
                            if use_b1:
                                bias_mm(qa, g, 0, RA, g * RA)
                        for g in range(4):
                            rec_mm(qb, g, RA, RB, g * RB, h_in, wt, True, False)
                            in_mm(qb, g, RA, RB, g * RB, xt, False, False,
                                  not use_b1)
                            if use_b1:
                                bias_mm(qb, g, RA, RB, g * RB)
                    return qa, qb

                def layer_sigs(qa, qb):
                    """2 sigmoids -> s [H, 2560] f32, gate-major contiguous."""
                    s = sigp.tile([H, 4 * R], f32, tag="s")
                    s3 = s[:].rearrange("p (g c) -> p g c", g=4)
                    a3 = qa[:].rearrange("p (g c) -> p g c", g=4)
                    nc.scalar.activation(s3[:, :, 0:RA], a3, AF.Sigmoid)
                    b3 = qb[:].rearrange("p (g c) -> p g c", g=4)
                    nc.scalar.activation(s3[:, :, RA:R], b3, AF.Sigmoid)
                    return s

                def cell_update(s, c_rd, c_wr):
                    """q = sf*c_prev; g~ = 2sg-1; p = si*g~; c = q + p.

                    Split at column RA: the A-slice path depends only on the
                    (earlier, bigger) quadA sigmoid, so c[0:RA] closes as
                    soon as sigA lands. Pool does q and g~; DVE does p, c."""
                    sf = s[:, 0:R]
                    sg = s[:, R : 2 * R]
                    si = s[:, 2 * R : 3 * R]
                    q = tmpp.tile([H, R], f32, tag="q")
                    gt = tmpp.tile([H, R], f32, tag="gt")
                    pbuf = tmpp.tile([H, R], f32, tag="p")
                    for cs in (slice(0, RA), slice(RA, R)):
                        nc.gpsimd.tensor_scalar(gt[:, cs], sg[:, cs], 2.0,
                                                -1.0, op0=ALU.mult,
                                                op1=ALU.add)
                        nc.gpsimd.tensor_mul(q[:, cs], sf[:, cs], c_rd[:, cs])
                        nc.vector.tensor_mul(pbuf[:, cs], si[:, cs],
                                             gt[:, cs])
                        nc.vector.tensor_add(c_wr[:, cs], pbuf[:, cs],
                                             q[:, cs])

                hist_tiles = []
                pend1 = None  # (s1, keep) awaiting tanh of c1(t-1)
                for t in range(scan_steps):
                    xt = xin.tile([128, R], bf16, tag="x")
                    for g in range(4):
                        nc.sync.dma_start(
                            out=xt[32 * g : 32 * g + KX, :],
                            in_=xrep_d[t, KX * g : KX * g + KX, :],
                        )

                    cp_next = cpp.tile([H, 2 * R], f32, tag="cp",
                                       name=f"cp{t+1}")

                    # ---- L0(t) ----
                    qa0, qb0 = layer_mms(xt, h0, whh0t, True)
                    s0 = layer_sigs(qa0, qb0)
                    cell_update(s0, cp_prev[:, 0:R], cp_cur[:, 0:R])

                    # tanh over pair(t) = [c0(t) | c1(t-1)], split at RA so
                    # h0[0:RA] unblocks the next layer's quadA matmuls early
                    tcpair = tcp.tile([H, 2 * R], f16, tag="tc")
                    so0 = s0[:, 3 * R : 4 * R]
                    nc.scalar.activation(tcpair[:, 0:RA], cp_cur[:, 0:RA],
                                         AF.Tanh)
                    h0 = h0p.tile([H, R], bf16, tag="h0")
                    nc.vector.tensor_mul(h0[:, 0:RA], so0[:, 0:RA],
                                         tcpair[:, 0:RA])
                    if t == 0:
                        nc.scalar.activation(tcpair[:, RA:R],
                                             cp_cur[:, RA:R], AF.Tanh)
                    else:
                        nc.scalar.activation(tcpair[:, RA : 2 * R],
                                             cp_cur[:, RA : 2 * R], AF.Tanh)
                    nc.vector.tensor_mul(h0[:, RA:R], so0[:, RA:R],
                                         tcpair[:, RA:R])

                    # L1 tail of step t-1: h1(t-1) = so1(t-1) * tanh(c1(t-1))
                    if pend1 is not None:
                        ps1, keep_prev = pend1
                        pool = histp if keep_prev else h1p
                        h1 = pool.tile([H, R], bf16,
                                       tag="hist" if keep_prev else "h1")
                        nc.vector.tensor_mul(h1[:], ps1[:, 3 * R : 4 * R],
                                             tcpair[:, R : 2 * R])
                        if keep_prev:
                            hist_tiles.append(h1)

                    # ---- L1(t) ----
                    qa1, qb1 = layer_mms(h0, h1, whh1t, False)
                    s1 = layer_sigs(qa1, qb1)
                    cell_update(s1, cp_cur[:, R : 2 * R],
                                cp_next[:, R : 2 * R])
                    pend1 = (s1, t >= scan_steps - n_hist)
                    cp_prev, cp_cur = cp_cur, cp_next

                # final L1 tail: tanh of c1(T-1) = cp_cur[R:2R]
                if pend1 is not None:
                    ps1, keep_prev = pend1
                    tcl = tcp.tile([H, R], f16, tag="tcl")
                    nc.scalar.activation(tcl[:], cp_cur[:, R : 2 * R], AF.Tanh)
                    pool = histp if keep_prev else h1p
                    h1 = pool.tile([H, R], bf16,
                                   tag="hist" if keep_prev else "h1")
                    nc.vector.tensor_mul(h1[:], ps1[:, 3 * R : 4 * R], tcl[:])
                    if keep_prev:
                        hist_tiles.append(h1)

                # ---- head ----
                stage = stagep.tile([64, R], f32)
                for s, ht in enumerate(hist_tiles):
                    hr = tmpp.tile([H, R], bf16, tag="hr")
                    nc.vector.tensor_scalar_max(hr[:], ht[:], 0.0)
                    hp = gq.tile([2, R], f32, tag="quad", name="hp")
                    for lo, w in ((0, 512), (512, 128)):
                        nc.tensor.matmul(
                            hp[:, lo : lo + w],
                            lhsT=wht[:, 0:2],
                            rhs=hr[:, lo : lo + w],
                            start=True,
                            stop=False,
                        )
                        nc.tensor.matmul(
                            hp[:, lo : lo + w],
                            lhsT=bh2t[0:1, 0:2],
                            rhs=ones[0:1, lo : lo + w],
                            start=False,
                            stop=True,
                        )
                    hs = tmpp.tile([2, R], f32, tag="hs")
                    if s % 2 == 0:
                        nc.scalar.copy(hs[:], hp[:])
                    else:
                        nc.vector.tensor_copy(hs[:], hp[:])
                    nc.sync.dma_start(out=stage[s : s + 1, :], in_=hs[0:1, :])
                    nc.sync.dma_start(
                        out=stage[32 + s : 32 + s + 1, :], in_=hs[1:2, :]
                    )

                sg = stage[32 : 32 + n_hist, :]
                nc.scalar.activation(sg, sg, AF.Exp)
                nc.vector.tensor_scalar_add(sg, sg, 1.0)
                nc.scalar.activation(sg, sg, AF.Ln)
                nc.sync.dma_start(out=out_d[:, 0, :], in_=stage[0:n_hist, :])
                nc.sync.dma_start(out=out_d[:, 1, :], in_=stage[32 : 32 + n_hist, :])

            if repeat > 1:
                with tc.For_i(0, repeat, 1):
                    _scan_body()
            else:
                _scan_body()

    nc.compile()
    return nc


def _prepare_inputs(inputs: dict, t_steps: int):
    import ml_dtypes

    bf = ml_dtypes.bfloat16
    perm = _gate_perm()
    hist = np.asarray(inputs["history_data"], np.float32)
    fut = np.asarray(inputs["future_data"], np.float32)
    We = np.asarray(inputs["We"], np.float32)
    be = np.asarray(inputs["be"], np.float32)
    Wih0 = np.asarray(inputs["Wih0"], np.float32)
    Whh0 = np.asarray(inputs["Whh0"], np.float32).copy()
    bih0 = np.asarray(inputs["bih0"], np.float32)
    bhh0 = np.asarray(inputs["bhh0"], np.float32)
    Wih1 = np.asarray(inputs["Wih1"], np.float32).copy()
    Whh1 = np.asarray(inputs["Whh1"], np.float32).copy()
    bih1 = np.asarray(inputs["bih1"], np.float32)
    bhh1 = np.asarray(inputs["bhh1"], np.float32)
    Wh = np.asarray(inputs["Wh"], np.float32)
    bh = np.asarray(inputs["bh"], np.float32)

    tgt = np.concatenate([hist[..., 0], fut[..., 0]], axis=1)      # [B, 120, N]
    cov = np.concatenate([hist[..., 1:], fut[..., 1:]], axis=1)    # [B, 120, N, COV]
    x6 = np.concatenate(
        [
            tgt[:, :t_steps, :, None],
            cov[:, 1 : t_steps + 1],
            np.ones((B, t_steps, N_SER, 1), np.float32),
        ],
        axis=-1,
    )  # [B, T, N, 6]
    x6 = x6.transpose(1, 0, 2, 3).reshape(t_steps, BN, KX)

    b0 = bih0 + bhh0 + Wih0[:, :E] @ be
    b1 = (bih1 + bhh1).copy()
    W0eff = np.concatenate(
        [Wih0[:, :E] @ We, Wih0[:, E:], b0[:, None]], axis=1
    )  # [512, 6]

    # scale the g-gate rows (torch block 2) by 2: tanh(g) = 2*sig(2g) - 1
    gsl = slice(2 * H, 3 * H)
    for w in (W0eff, Whh0, Wih1, Whh1):
        w[gsl] *= 2.0
    b1[gsl] *= 2.0

    W0r = W0eff[perm]
    b1r = b1[perm]
    whh0t = np.ascontiguousarray(Whh0[perm].T).astype(bf)   # [128, 512]
    wih1t = np.ascontiguousarray(Wih1[perm].T).astype(bf)
    whh1t = np.ascontiguousarray(Whh1[perm].T).astype(bf)

    w0rep = np.zeros((128, 128), np.float32)
    w0t = W0r.T  # [6, 512]
    for g in range(4):
        w0rep[32 * g : 32 * g + KX, :] = w0t[:, g * H : (g + 1) * H]

    with_b1 = bool(np.any(b1r != 0))
    shared = {
        "whh0t": whh0t,
        "w0rep": w0rep.astype(bf),
        "wih1t": wih1t,
        "whh1t": whh1t,
        "wht": np.ascontiguousarray(Wh.T).astype(bf),
        "bh2": bh.reshape(1, 2).astype(bf),
    }
    if with_b1:
        shared["b1r"] = b1r.reshape(1, G4).astype(bf)
    in_maps = []
    for c in range(NCORES):
        xc = x6[:, c * R : (c + 1) * R, :]           # [T, R, 6]
        xt = np.ascontiguousarray(xc.transpose(0, 2, 1))  # [T, 6, R]
        xrep = np.tile(xt, (1, 4, 1))                # [T, 24, R]
        in_maps.append({"xrep": np.ascontiguousarray(xrep).astype(bf), **shared})
    return in_maps


def kernel(**inputs) -> np.ndarray:
    from concourse.bass_utils import run_bass_kernel_spmd

    t_steps = int(os.environ.get("DEEPAR_T_STEPS", T_STEPS))
    in_maps = _prepare_inputs(inputs, t_steps)
    with_b1 = "b1r" in in_maps[0]
    key = (t_steps, with_b1)
    if key not in _PROGRAM_CACHE:
        _PROGRAM_CACHE[key] = _build_program(t_steps, with_b1=with_b1)
    nc = _PROGRAM_CACHE[key]

    res = run_bass_kernel_spmd(nc, in_maps, list(range(NCORES)))
    outs = [np.asarray(r["out"], np.float32) for r in res.results]
    full = np.concatenate(outs, axis=2)  # [n_hist, 2, BN]
    n_hist = full.shape[0]
    return np.ascontiguousarray(
        full.reshape(n_hist, 2, B, N_SER).transpose(2, 0, 3, 1)
    ).astype(np.float32)


# revision 32
# speedup vs baseline: 2.4442x; 2.4442x over previous
"""DeepAR 2-layer LSTM (B=16, L_IN=96, L_OUT=24, N=320, H=128) on 8 TRN2 cores.

Strategy (data-parallel over B*N=5120 rows, 640 rows/core):
  - Layout: hidden/gate index on SBUF partitions, batch rows on the free dim.
    h, c are [128, 640] tiles; gates are computed as W.T-slices (lhsT) against
    h (rhs) so no transposes are ever needed.
  - The input-side transform of layer 0 is folded through the embedding:
    x_eff[t] = [tgt_t, cov_{t+1}] (5 values/row) with W0eff = [Wih0[:, :E] @ We,
    Wih0[:, E:]] -> K=5 matmuls, packed 4-way with tile_position row strips.
  - Gate order is permuted to (i, f, o, g) host-side.
  - bf16 matmul operands (1 cyc/row on PE); cell state c and all gate
    nonlinearities in fp32.
  - Engines: PE matmuls; ACT sigmoid/tanh (bottleneck ~7us/step); DVE does
    p=si*tg, c=p+q, h=so*tc; GPSIMD does q=sf*c_prev.
  - Head (last 24 steps) runs post-scan from h1 history kept in SBUF; softplus
    on device; bias folded in via a K=1 matmul against a ones row.
"""

import os

import numpy as np

B, L_IN, L_OUT, N_SER, COV = 16, 96, 24, 320, 4
E, H = 64, 128
T_STEPS = 119  # L_IN + L_OUT - 1
NCORES = 8
BN = B * N_SER          # 5120
R = BN // NCORES        # 640 rows per core
G4 = 4 * H              # 512 gates

_PROGRAM_CACHE: dict = {}


def _gate_perm() -> np.ndarray:
    # torch gate order in weights: i, f, g, o -> reorder rows to (f, g, i, o)
    # so the critical-path gates (forget, cell) come out of the PE first
    idx = np.arange(G4).reshape(4, H)
    return np.concatenate([idx[1], idx[2], idx[0], idx[3]])


def _build_program(t_steps: int, scan_steps: int | None = None, repeat: int = 1):
    import concourse.bacc as bacc
    import concourse.mybir as mybir
    import concourse.tile as tile

    f32 = mybir.dt.float32
    bf16 = mybir.dt.bfloat16
    f16 = mybir.dt.float16
    AF = mybir.ActivationFunctionType

    nc = bacc.Bacc()

    xrep_d = nc.declare_dram_parameter("xrep", [t_steps, 20, R], bf16, isOutput=False)
    whh0t_d = nc.declare_dram_parameter("whh0t", [H, G4], bf16, isOutput=False)
    w0rep_d = nc.declare_dram_parameter("w0rep", [128, 128], bf16, isOutput=False)
    wih1t_d = nc.declare_dram_parameter("wih1t", [H, G4], bf16, isOutput=False)
    whh1t_d = nc.declare_dram_parameter("whh1t", [H, G4], bf16, isOutput=False)
    b0_d = nc.declare_dram_parameter("b0", [H, 4], f32, isOutput=False)
    b1_d = nc.declare_dram_parameter("b1", [H, 4], f32, isOutput=False)
    wht_d = nc.declare_dram_parameter("wht", [H, 2], bf16, isOutput=False)
    bh2_d = nc.declare_dram_parameter("bh2", [1, 2], bf16, isOutput=False)
    if scan_steps is None:
        scan_steps = t_steps
    n_hist = min(L_OUT, scan_steps)  # h1 steps kept for the head
    out_d = nc.declare_dram_parameter("out", [n_hist, 2, R], f32, isOutput=True)

    with tile.TileContext(nc) as tc:
        with (
            tc.tile_pool(name="consts", bufs=1) as consts,
            tc.tile_pool(name="xin", bufs=5) as xin,
            tc.tile_pool(name="gates", bufs=4, space="PSUM") as gpsum,
            tc.tile_pool(name="sig", bufs=12) as sigp,
            tc.tile_pool(name="tmp", bufs=6) as tmpp,
            tc.tile_pool(name="tcp", bufs=6) as tcp,
            tc.tile_pool(name="h0p", bufs=4) as h0p,
            tc.tile_pool(name="c0p", bufs=3) as c0p,
            tc.tile_pool(name="c1p", bufs=3) as c1p,
            tc.tile_pool(name="h1p", bufs=4) as h1p,
            tc.tile_pool(name="hist", bufs=n_hist + 1) as histp,
            tc.tile_pool(name="stage", bufs=1) as stagep,
        ):
            # ---- load constants ----
            whh0t = consts.tile([H, G4], bf16)
            nc.sync.dma_start(out=whh0t[:], in_=whh0t_d[:])
            w0rep = consts.tile([128, 128], bf16)
            nc.sync.dma_start(out=w0rep[:], in_=w0rep_d[:])
            wih1t = consts.tile([H, G4], bf16)
            nc.sync.dma_start(out=wih1t[:], in_=wih1t_d[:])
            whh1t = consts.tile([H, G4], bf16)
            nc.sync.dma_start(out=whh1t[:], in_=whh1t_d[:])
            b0t = consts.tile([H, 4], f32)
            nc.sync.dma_start(out=b0t[:], in_=b0_d[:])
            b1t = consts.tile([H, 4], f32)
            nc.sync.dma_start(out=b1t[:], in_=b1_d[:])
            wht = consts.tile([H, 2], bf16)
            nc.sync.dma_start(out=wht[:], in_=wht_d[:])
            bh2t = consts.tile([1, 2], bf16)
            nc.sync.dma_start(out=bh2t[:], in_=bh2_d[:])
            ones = consts.tile([1, R], bf16)
            nc.vector.memset(ones[:], 1.0)

            def _scan_body():
                # ---- initial states ----
                h0 = h0p.tile([H, R], bf16)
                nc.vector.memset(h0[:], 0.0)
                h1 = h1p.tile([H, R], bf16)
                nc.vector.memset(h1[:], 0.0)
                c0 = c0p.tile([H, R], f16)
                nc.vector.memset(c0[:], 0.0)
                c1 = c1p.tile([H, R], f16)
                nc.vector.memset(c1[:], 0.0)

                CH = [(0, 512), (512, 128)]  # psum-bank-aligned column chunks

                def layer_head(gates_mm, bt, c_rd, c_wr):
                    """MMs + sigmoids + p/q/c-update for one layer-step."""
                    gp = [
                        gpsum.tile([H, R], f32, tag="gates", name=f"gp{g}")
                        for g in range(4)
                    ]
                    for g in range(4):
                        gates_mm(gp[g], g)
                    sf = sigp.tile([H, R], f16, tag="sig")
                    nc.scalar.activation(sf[:], gp[0][:], AF.Sigmoid, bias=bt[:, 0:1])
                    q = tmpp.tile([H, R], f16, tag="tmp")
                    nc.gpsimd.tensor_mul(q[:], sf[:], c_rd)
                    tg = sigp.tile([H, R], f16, tag="sig")
                    nc.scalar.activation(tg[:], gp[1][:], AF.Tanh, bias=bt[:, 1:2])
                    si = sigp.tile([H, R], f16, tag="sig")
                    nc.scalar.activation(si[:], gp[2][:], AF.Sigmoid, bias=bt[:, 2:3])
                    # so/tc in bf16: they only feed h (already bf16) and unlock
                    # the DVE 2x mode for the h-multiply on the critical path
                    so = sigp.tile([H, R], f16, tag="sigb")
                    nc.scalar.activation(so[:], gp[3][:], AF.Sigmoid, bias=bt[:, 3:4])
                    p = tmpp.tile([H, R], f16, tag="tmp")
                    for lo, w in CH:
                        s = slice(lo, lo + w)
                        nc.vector.tensor_mul(p[:, s], si[:, s], tg[:, s])
                        nc.vector.tensor_add(c_wr[:, lo : lo + w], p[:, s],
                                             q[:, s])
                    return so

                def layer_tail(c_ap, so, hpool, h_hist_pool=None,
                               split=True):
                    """tanh(c) and h = so*tanh(c); split at the bank
                    boundary so h[0:512] unblocks the next matmuls early.
                    The early L1-tail (c1 ready since last step) runs
                    unsplit: one 640-wide tanh costs less ACT overhead."""
                    tcv = tcp.tile([H, R], f16, tag="tc")
                    pool = h_hist_pool if h_hist_pool is not None else hpool
                    h_new = pool.tile([H, R], bf16, tag="hist" if h_hist_pool else None)
                    chunks = CH if split else [(0, R)]
                    for lo, w in chunks:
                        s = slice(lo, lo + w)
                        nc.scalar.activation(tcv[:, s], c_ap[:, lo : lo + w],
                                             AF.Tanh)
                        nc.vector.tensor_mul(h_new[:, s], so[:, s], tcv[:, s])
                    return h_new

                hist_tiles = []
                h0_entry = h0  # h0(t-1) for the trailing L1 phase
                h1_entry = h1
                for t in range(scan_steps):
                    # x input for this step: [20, R] replicated 4x at partition
                    # offsets 0/32/64/96, 5 rows each
                    xt = xin.tile([128, R], bf16, tag="x")
                    for g in range(4):
                        nc.sync.dma_start(
                            out=xt[32 * g : 32 * g + 5, :],
                            in_=xrep_d[t, 5 * g : 5 * g + 5, :],
                        )

                    def l0_mm(gp, g, xt=xt, h0=h0):
                        # x-part first: xt is available long before h0(t-1)
                        for lo, w in CH:
                            nc.tensor.matmul(
                                gp[:, lo : lo + w],
                                lhsT=w0rep[32 * g : 32 * g + 5, :],
                                rhs=xt[32 * g : 32 * g + 5, lo : lo + w],
                                start=True,
                                stop=False,
                                tile_position=(32 * g, 0),
                            )
                        for lo, w in CH:
                            nc.tensor.matmul(
                                gp[:, lo : lo + w],
                                lhsT=whh0t[:, g * H : (g + 1) * H],
                                rhs=h0[:, lo : lo + w],
                                start=False,
                                stop=True,
                            )

                    c0_new = c0p.tile([H, R], f16, tag="c0")
                    so0 = layer_head(l0_mm, b0t, c0[:], c0_new[:])
                    h0_entry = h0
                    h0 = layer_tail(c0_new[:], so0, h0p, split=False)
                    c0 = c0_new

                    # ---- trailing L1 phase for step t-1: all operands
                    # (h0(t-1), h1(t-2)) are a step old -> never blocks ----
                    if t == 0:
                        continue
                    tl = t - 1

                    def l1_mm(gp, g, h0e=h0_entry, h1e=h1):
                        # recurrent part first (h1(t-2) oldest)
                        for lo, w in CH:
                            nc.tensor.matmul(
                                gp[:, lo : lo + w],
                                lhsT=whh1t[:, g * H : (g + 1) * H],
                                rhs=h1e[:, lo : lo + w],
                                start=True,
                                stop=False,
                            )
                        for lo, w in CH:
                            nc.tensor.matmul(
                                gp[:, lo : lo + w],
                                lhsT=wih1t[:, g * H : (g + 1) * H],
                                rhs=h0e[:, lo : lo + w],
                                start=False,
                                stop=True,
                            )

                    keep = tl >= scan_steps - n_hist
                    c1_new = c1p.tile([H, R], f16, tag="c1")
                    so1 = layer_head(l1_mm, b1t, c1[:], c1_new[:])
                    h1 = layer_tail(c1_new[:], so1, h1p,
                                    histp if keep else None, split=False)
                    if keep:
                        hist_tiles.append(h1)
                    c1 = c1_new

                # final trailing L1 phase for t = scan_steps-1
                tl = scan_steps - 1

                def l1_mm_last(gp, g, h0e=h0, h1e=h1):
                    for lo, w in CH:
                        nc.tensor.matmul(
                            gp[:, lo : lo + w],
                            lhsT=whh1t[:, g * H : (g + 1) * H],
                            rhs=h1e[:, lo : lo + w],
                            start=True,
                            stop=False,
                        )
                    for lo, w in CH:
                        nc.tensor.matmul(
                            gp[:, lo : lo + w],
                            lhsT=wih1t[:, g * H : (g + 1) * H],
                            rhs=h0e[:, lo : lo + w],
                            start=False,
                            stop=True,
                        )

                keep = tl >= scan_steps - n_hist
                c1_new = c1p.tile([H, R], f16, tag="c1")
                so1 = layer_head(l1_mm_last, b1t, c1[:], c1_new[:])
                h1 = layer_tail(c1_new[:], so1, h1p,
                                histp if keep else None, split=False)
                if keep:
                    hist_tiles.append(h1)

                # ---- head: mu/sigma for the last L_OUT steps ----
                stage = stagep.tile([64, R], f32)  # mu rows 0..n_hist-1, sigma rows 32..
                for s, ht in enumerate(hist_tiles):
                    hr = tmpp.tile([H, R], bf16, tag="hr")
                    nc.vector.tensor_scalar_max(hr[:], ht[:], 0.0)
                    hp = gpsum.tile([2, R], f32, tag="gates", name="hp")
                    for lo, w in CH:
                        nc.tensor.matmul(
                            hp[:, lo : lo + w],
                            lhsT=wht[:, 0:2],
                            rhs=hr[:, lo : lo + w],
                            start=True,
                            stop=False,
                        )
                        nc.tensor.matmul(
                            hp[:, lo : lo + w],
                            lhsT=bh2t[0:1, 0:2],
                            rhs=ones[0:1, lo : lo + w],
                            start=False,
                            stop=True,
                        )
                    hs = tmpp.tile([2, R], f32, tag="hs")
                    if s % 2 == 0:
                        nc.scalar.copy(hs[:], hp[:])
                    else:
                        nc.vector.tensor_copy(hs[:], hp[:])
                    nc.sync.dma_start(out=stage[s : s + 1, :], in_=hs[0:1, :])
                    nc.sync.dma_start(
                        out=stage[32 + s : 32 + s + 1, :], in_=hs[1:2, :]
                    )

                # softplus(x) = ln(1 + exp(x)); Softplus has no ACT table in this
                # compiler build. Head preacts are small, so exp cannot overflow.
                sg = stage[32 : 32 + n_hist, :]
                nc.scalar.activation(sg, sg, AF.Exp)
                nc.vector.tensor_scalar_add(sg, sg, 1.0)
                nc.scalar.activation(sg, sg, AF.Ln)
                nc.sync.dma_start(out=out_d[:, 0, :], in_=stage[0:n_hist, :])
                nc.sync.dma_start(out=out_d[:, 1, :], in_=stage[32 : 32 + n_hist, :])


            if repeat > 1:
                with tc.For_i(0, repeat, 1):
                    _scan_body()
            else:
                _scan_body()

    nc.compile()
    return nc


def _prepare_inputs(inputs: dict, t_steps: int):
    import ml_dtypes

    bf = ml_dtypes.bfloat16
    perm = _gate_perm()
    hist = np.asarray(inputs["history_data"], np.float32)
    fut = np.asarray(inputs["future_data"], np.float32)
    We = np.asarray(inputs["We"], np.float32)
    be = np.asarray(inputs["be"], np.float32)
    Wih0 = np.asarray(inputs["Wih0"], np.float32)
    Whh0 = np.asarray(inputs["Whh0"], np.float32)
    bih0 = np.asarray(inputs["bih0"], np.float32)
    bhh0 = np.asarray(inputs["bhh0"], np.float32)
    Wih1 = np.asarray(inputs["Wih1"], np.float32)
    Whh1 = np.asarray(inputs["Whh1"], np.float32)
    bih1 = np.asarray(inputs["bih1"], np.float32)
    bhh1 = np.asarray(inputs["bhh1"], np.float32)
    Wh = np.asarray(inputs["Wh"], np.float32)
    bh = np.asarray(inputs["bh"], np.float32)

    tgt = np.concatenate([hist[..., 0], fut[..., 0]], axis=1)      # [B, 120, N]
    cov = np.concatenate([hist[..., 1:], fut[..., 1:]], axis=1)    # [B, 120, N, COV]
    x5 = np.concatenate(
        [tgt[:, :t_steps, :, None], cov[:, 1 : t_steps + 1]], axis=-1
    )  # [B, T, N, 5]
    x5 = x5.transpose(1, 0, 2, 3).reshape(t_steps, BN, 5)

    W0eff = np.concatenate([Wih0[:, :E] @ We, Wih0[:, E:]], axis=1)  # [512, 5]
    b0 = bih0 + bhh0 + Wih0[:, :E] @ be
    b1 = bih1 + bhh1

    W0r = W0eff[perm]
    b0r = b0[perm]
    b1r = b1[perm]
    whh0t = np.ascontiguousarray(Whh0[perm].T).astype(bf)   # [128, 512]
    wih1t = np.ascontiguousarray(Wih1[perm].T).astype(bf)
    whh1t = np.ascontiguousarray(Whh1[perm].T).astype(bf)

    w0rep = np.zeros((128, 128), np.float32)
    w0t = W0r.T  # [5, 512]
    for g in range(4):
        w0rep[32 * g : 32 * g + 5, :] = w0t[:, g * H : (g + 1) * H]

    shared = {
        "whh0t": whh0t,
        "w0rep": w0rep.astype(bf),
        "wih1t": wih1t,
        "whh1t": whh1t,
        "b0": np.ascontiguousarray(b0r.reshape(4, H).T),
        "b1": np.ascontiguousarray(b1r.reshape(4, H).T),
        "wht": np.ascontiguousarray(Wh.T).astype(bf),
        "bh2": bh.reshape(1, 2).astype(bf),
    }
    in_maps = []
    for c in range(NCORES):
        xc = x5[:, c * R : (c + 1) * R, :]           # [T, R, 5]
        xt = np.ascontiguousarray(xc.transpose(0, 2, 1))  # [T, 5, R]
        xrep = np.tile(xt, (1, 4, 1))                # [T, 20, R]
        in_maps.append({"xrep": np.ascontiguousarray(xrep).astype(bf), **shared})
    return in_maps


def kernel(**inputs) -> np.ndarray:
    from concourse.bass_utils import run_bass_kernel_spmd

    t_steps = int(os.environ.get("DEEPAR_T_STEPS", T_STEPS))
    if t_steps not in _PROGRAM_CACHE:
        _PROGRAM_CACHE[t_steps] = _build_program(t_steps)
    nc = _PROGRAM_CACHE[t_steps]

    in_maps = _prepare_inputs(inputs, t_steps)
    res = run_bass_kernel_spmd(nc, in_maps, list(range(NCORES)))
    outs = [np.asarray(r["out"], np.float32) for r in res.results]
    full = np.concatenate(outs, axis=2)  # [n_hist, 2, BN]
    n_hist = full.shape[0]
    return np.ascontiguousarray(
        full.reshape(n_hist, 2, B, N_SER).transpose(2, 0, 3, 1)
    ).astype(np.float32)

